# revision 17
# baseline (speedup 1.0000x reference)
"""Tensor-parallel GQA attention layer (T=2048, dim=4096, 32 q-heads / 8 kv-heads,
D=128, interleaved RoPE, causal) for 8 Trainium2 NeuronCores.

Sharding: TP over heads. Each core owns 4 q-heads + 1 kv-head:
  - w_qkv rows (head-grouped) sharded -> per-core [768, 4096]
  - w_o columns sharded -> per-core [4096, 512]
  - x replicated
Each core computes its partial output [2048, 4096] in fp32; the host sums the
8 partials (equivalent to the all-reduce) and casts to bf16.

Device layout (per core) -- everything kept "transposed" so each stage's
output is the next stage's natural PE operand:
  qkv^T [768, 2048] = (w_qkvT tiles).T @ xT tiles          (j on partitions)
  rope on q/k tiles via pair-swap permutation matmul + elementwise tables
  S^T [s, t] = KrT_tile.T @ QrT  -> exp on ACT -> P^T bf16 (causal: skip/mask)
  r[t] = ones.T @ P^T (row sums, broadcast across partitions)
  U^T [d, t] = V_tile.T-accumulated PV; AO^T = U^T * (1/r)
  OUT [t, m] = AO^T_tile.T @ w_oT -> fp32 partial to DRAM
"""
import numpy as np
import ml_dtypes

T, DIM, H, HKV, D, NCORES = 2048, 4096, 32, 8, 128, 8
HL = H // NCORES            # 4 local q heads
JL = (HL + 2) * D           # 768 local qkv rows
WO_L = HL * D               # 512 local w_o cols
SCALE = float(D) ** -0.5
THETA = 10000.0
NP_BF16 = ml_dtypes.bfloat16

_CACHE = {}


def _build_nc(reps=1):
    from contextlib import ExitStack
    import concourse.bacc as bacc
    import concourse.mybir as mybir
    from concourse.tile import TileContext
    from concourse.masks import make_identity

    bf = mybir.dt.bfloat16
    f32 = mybir.dt.float32
    Exp = mybir.ActivationFunctionType.Exp

    nc = bacc.Bacc("TRN2", target_bir_lowering=False, debug=False,
                   num_devices=NCORES)
    xT_h = nc.dram_tensor("xT", [DIM, T], bf, kind="ExternalInput")
    wq_h = nc.dram_tensor("wqkvT", [DIM, JL], bf, kind="ExternalInput")
    wo_h = nc.dram_tensor("woT", [WO_L, DIM], bf, kind="ExternalInput")
    cos_h = nc.dram_tensor("cosb", [D, T], f32, kind="ExternalInput")
    sin_h = nc.dram_tensor("sinb", [D, T], f32, kind="ExternalInput")
    psw_h = nc.dram_tensor("pswap", [D, D], bf, kind="ExternalInput")
    msk_h = nc.dram_tensor("mask01", [D, D], bf, kind="ExternalInput")
    out_h = nc.dram_tensor("outp", [T, DIM], f32, kind="ExternalOutput")
    xT, wq, wo = xT_h.ap(), wq_h.ap(), wo_h.ap()
    cosb, sinb, psw, msk, outp = cos_h.ap(), sin_h.ap(), psw_h.ap(), msk_h.ap(), out_h.ap()

    def emit_once(tc, top):
        const = top.enter_context(tc.tile_pool(name="const", bufs=1))
        pswap_sb = const.tile([D, D], bf, name="pswap_sb")
        mask_sb = const.tile([D, D], bf, name="mask_sb")
        ones_sb = const.tile([D, D], bf, name="ones_sb")
        ident_sb = const.tile([D, D], bf, name="ident_sb")
        nc.sync.dma_start(pswap_sb[:], psw[:, :])
        nc.sync.dma_start(mask_sb[:], msk[:, :])
        nc.vector.memset(ones_sb[:], 1.0)
        make_identity(nc, ident_sb[:])

        persist = top.enter_context(tc.tile_pool(name="persist", bufs=1))
        # Qr^T per local head + Kr^T: bf16 [128, T]
        rot_sb = [persist.tile([D, T], bf, name=f"rot{jt}", tag=f"rot{jt}")
                  for jt in range(HL + 1)]
        # V natural orientation, one [128, 128] tile per s-block
        v_sb = [persist.tile([D, D], bf, name=f"v{st}", tag=f"v{st}")
                for st in range(T // D)]
        # AO^T per local head
        ao_sb = [persist.tile([D, T], bf, name=f"ao{h}", tag=f"ao{h}")
                 for h in range(HL)]

        # ---------------- phase 1: QKV + RoPE + V transpose ----------------
        with ExitStack() as ph1:
            wp = ph1.enter_context(tc.tile_pool(name="wp", bufs=1))
            xp = ph1.enter_context(tc.tile_pool(name="xp", bufs=1))
            rp = ph1.enter_context(tc.tile_pool(name="rp", bufs=1))
            sp = ph1.enter_context(tc.tile_pool(name="sp", bufs=2))
            tp = ph1.enter_context(tc.tile_pool(name="tp", bufs=4))
            qps = ph1.enter_context(tc.tile_pool(name="qps", bufs=2, space="PSUM"))
            sps = ph1.enter_context(tc.tile_pool(name="sps", bufs=2, space="PSUM"))
            vps = ph1.enter_context(tc.tile_pool(name="vps", bufs=2, space="PSUM"))

            TH = 1024  # t-half width
            # DMA order matches PE consumption order: x[i] + w[jt=0][i] pairs
            # first (so jt=0 can stream while later wq columns still arrive),
            # then w for jt=1..5, then rope tables.
            # jt=0 weight columns as small tiles up front (paired with x so the
            # first QKV sweep streams without waiting on the full wq), the
            # remaining 5 column-tiles as one wide DMA per i afterwards.
            w_sb = []
            x0_sb = []
            for i in range(32):
                xt = xp.tile([D, TH], bf, name=f"x{i}", tag=f"x{i}")
                nc.sync.dma_start(xt[:], xT[i * 128:(i + 1) * 128, 0:TH])
                x0_sb.append(xt)
                wt = wp.tile([D, JL], bf, name=f"w{i}", tag=f"w{i}")
                nc.sync.dma_start(wt[:], wq[i * 128:(i + 1) * 128, :])
                w_sb.append(wt)
            cos_sb = rp.tile([D, T], f32, name="cos_sb")
            sin_sb = rp.tile([D, T], f32, name="sin_sb")
            nc.sync.dma_start(cos_sb[:], cosb[:, :])
            nc.sync.dma_start(sin_sb[:], sinb[:, :])

            for th in range(2):
                t0 = th * TH
                if th == 0:
                    x_sb = x0_sb
                else:
                    x_sb = []
                    for i in range(32):
                        xt = xp.tile([D, TH], bf, name=f"x{i}", tag=f"x{i}")
                        nc.sync.dma_start(xt[:], xT[i * 128:(i + 1) * 128, t0:t0 + TH])
                        x_sb.append(xt)
                for jt in range(6):
                    qkv_ps = qps.tile([D, TH], f32, name="qkv_ps", tag="qkv")
                    for i in range(32):
                        lhsT = w_sb[i][:, jt * 128:(jt + 1) * 128]
                        for t2 in range(2):
                            nc.tensor.matmul(
                                qkv_ps[:, t2 * 512:(t2 + 1) * 512], lhsT,
                                x_sb[i][:, t2 * 512:(t2 + 1) * 512],
                                start=(i == 0), stop=(i == 31))
                    qkv_bf = sp.tile([D, TH], bf, name="qkv_bf", tag="qkv_bf")
                    nc.vector.tensor_copy(qkv_bf[:], qkv_ps[:])
                    if jt < 5:
                        # rope: rot = qkv*cos + (P qkv)*sin_signed
                        for t2 in range(2):
                            sl = slice(t2 * 512, (t2 + 1) * 512)
                            gl = slice(t0 + t2 * 512, t0 + t2 * 512 + 512)
                            sw_ps = sps.tile([D, 512], f32, name="sw_ps", tag="sw")
                            nc.tensor.matmul(sw_ps[:], pswap_sb[:], qkv_bf[:, sl],
                                             start=True, stop=True)
                            t1 = tp.tile([D, 512], f32, name="t1", tag="t1")
                            t2t = tp.tile([D, 512], f32, name="t2t", tag="t2t")
                            nc.vector.tensor_mul(t1[:], qkv_bf[:, sl], cos_sb[:, gl])
                            nc.vector.tensor_mul(t2t[:], sw_ps[:], sin_sb[:, gl])
                            nc.vector.tensor_add(rot_sb[jt][:, gl], t1[:], t2t[:])
                    else:
                        # V: transpose [t, d] blocks out of v^T
                        for k in range(8):
                            st = th * 8 + k
                            vt_ps = vps.tile([D, D], bf, name="vt_ps", tag="vt")
                            nc.tensor.transpose(
                                vt_ps[:], qkv_bf[:, k * 128:(k + 1) * 128],
                                ident_sb[:])
                            nc.vector.tensor_copy(v_sb[st][:], vt_ps[:])

        # w_o prefetch pool opened before phase 2 so its DMAs overlap attention
        wop = top.enter_context(tc.tile_pool(name="wop", bufs=1))
        wo_sb = []
        for jc in range(HL):
            wt = wop.tile([D, DIM], bf, name=f"wo{jc}", tag=f"wo{jc}")
            nc.sync.dma_start(wt[:], wo[jc * 128:(jc + 1) * 128, :])
            wo_sb.append(wt)

        # ---------------- phase 2: causal attention per head ----------------
        with ExitStack() as ph2:
            ptp = ph2.enter_context(tc.tile_pool(name="ptp", bufs=20))
            rip = ph2.enter_context(tc.tile_pool(name="rip", bufs=2))
            scs = ph2.enter_context(tc.tile_pool(name="scs", bufs=4, space="PSUM"))
            rps = ph2.enter_context(tc.tile_pool(name="rps", bufs=2, space="PSUM"))
            ups = ph2.enter_context(tc.tile_pool(name="ups", bufs=2, space="PSUM"))

            for h in range(HL):
                for j in range(4):          # t-chunks of 512
                    n_st = 4 * j + 4
                    pts = []
                    for st in range(n_st):
                        t_off = max(0, st - 4 * j) * 128
                        sc = scs.tile([D, 512], f32, name="sc", tag="sc")
                        nc.tensor.matmul(
                            sc[:, t_off:], rot_sb[HL][:, st * 128:(st + 1) * 128],
                            rot_sb[h][:, j * 512 + t_off:(j + 1) * 512],
                            start=True, stop=True)
                        pt = ptp.tile([D, 512], bf, name="pt", tag="pt")
                        nc.scalar.activation(pt[:, t_off:], sc[:, t_off:], Exp,
                                             scale=SCALE)
                        if st >= 4 * j:
                            nc.vector.tensor_mul(pt[:, t_off:t_off + 128],
                                                 pt[:, t_off:t_off + 128],
                                                 mask_sb[:])
                        pts.append(pt)
                    r_ps = rps.tile([D, 512], f32, name="r_ps", tag="r")
                    u_ps = ups.tile([D, 512], f32, name="u_ps", tag="u")
                    for st in range(n_st):
                        t_off = max(0, st - 4 * j) * 128
                        nc.tensor.matmul(r_ps[:, t_off:], ones_sb[:],
                                         pts[st][:, t_off:],
                                         start=(st == 0), stop=(st == n_st - 1))
                    for st in range(n_st):
                        t_off = max(0, st - 4 * j) * 128
                        nc.tensor.matmul(u_ps[:, t_off:], v_sb[st][:],
                                         pts[st][:, t_off:],
                                         start=(st == 0), stop=(st == n_st - 1))
                    r_inv = rip.tile([D, 512], f32, name="r_inv", tag="ri")
                    nc.vector.reciprocal(r_inv[:], r_ps[:])
                    nc.vector.tensor_mul(ao_sb[h][:, j * 512:(j + 1) * 512],
                                         u_ps[:], r_inv[:])

        # ---------------- phase 3: output projection (partial) ----------------
        with ExitStack() as ph3:
            obp = ph3.enter_context(tc.tile_pool(name="obp", bufs=3))
            ops = ph3.enter_context(tc.tile_pool(name="ops", bufs=2, space="PSUM"))
            for tt in range(16):
                for half in range(2):
                    po = ops.tile([D, 2048], f32, name="po", tag="po")
                    for jc in range(HL):
                        lhsT = ao_sb[jc][:, tt * 128:(tt + 1) * 128]
                        for mb in range(4):
                            m0 = half * 2048 + mb * 512
                            nc.tensor.matmul(po[:, mb * 512:(mb + 1) * 512],
                                             lhsT, wo_sb[jc][:, m0:m0 + 512],
                                             start=(jc == 0), stop=(jc == HL - 1))
                    ob = obp.tile([D, 2048], f32, name="ob", tag="ob")
                    nc.vector.tensor_copy(ob[:], po[:])
                    nc.sync.dma_start(
                        outp[tt * 128:(tt + 1) * 128,
                             half * 2048:(half + 1) * 2048], ob[:])

    from contextlib import ExitStack as _ES
    with TileContext(nc) as tc:
        for _rep in range(reps):
            with _ES() as top:
                emit_once(tc, top)

    nc.compile()
    return nc


def get_nc(reps=1):
    key = ("nc", reps)
    if key not in _CACHE:
        _CACHE[key] = _build_nc(reps)
    return _CACHE[key]


def host_prep(x, w_qkv, w_o):
    """Returns per-core input maps (numpy)."""
    x = np.asarray(x)
    w_qkv = np.asarray(w_qkv)
    w_o = np.asarray(w_o)
    xT = np.ascontiguousarray(x.T)
    inv_freq = 1.0 / (THETA ** (np.arange(0, D, 2, dtype=np.float64) / D))
    ang = np.arange(T, dtype=np.float64)[:, None] * inv_freq[None, :]
    cosb = np.empty((D, T), np.float32)
    sinb = np.empty((D, T), np.float32)
    cosb[0::2] = np.cos(ang).T
    cosb[1::2] = np.cos(ang).T
    sinb[0::2] = -np.sin(ang).T
    sinb[1::2] = np.sin(ang).T
    pswap = np.zeros((D, D), NP_BF16)
    for d in range(D):
        pswap[d, d ^ 1] = 1
    mask01 = np.triu(np.ones((128, 128), np.float32)).astype(NP_BF16)
    in_maps = []
    for c in range(NCORES):
        wq_rows = w_qkv[c * HL * D:(c + 1) * HL * D]
        wk_rows = w_qkv[H * D + c * D: H * D + (c + 1) * D]
        wv_rows = w_qkv[(H + HKV) * D + c * D:(H + HKV) * D + (c + 1) * D]
        w_c = np.concatenate([wq_rows, wk_rows, wv_rows], axis=0)
        in_maps.append({
            "xT": xT,
            "wqkvT": np.ascontiguousarray(w_c.T),
            "woT": np.ascontiguousarray(w_o[:, c * WO_L:(c + 1) * WO_L].T),
            "cosb": cosb, "sinb": sinb, "pswap": pswap, "mask01": mask01,
        })
    return in_maps


def kernel(x, w_qkv, w_o):
    from concourse.bass_utils import run_bass_kernel_spmd
    nc = get_nc()
    in_maps = host_prep(x, w_qkv, w_o)
    res = run_bass_kernel_spmd(nc, in_maps, list(range(NCORES)))
    acc = np.zeros((T, DIM), np.float32)
    for c in range(NCORES):
        acc += res.results[c]["outp"]
    return acc.astype(NP_BF16)


# revision 23
# speedup vs baseline: 1.5900x; 1.5900x over previous
"""Tensor-parallel GQA attention layer (T=2048, dim=4096, 32 q-heads / 8 kv-heads,
D=128, interleaved RoPE, causal) for 8 Trainium2 NeuronCores.

Sharding: TP over heads. Each core owns 4 q-heads + 1 kv-head:
  - w_qkv rows (head-grouped) sharded -> per-core [768, 4096]
  - w_o columns sharded -> per-core [4096, 512]
  - x replicated
Each core computes its partial output [2048, 4096] in fp32; the host sums the
8 partials (equivalent to the all-reduce) and casts to bf16.

Device layout (per core) -- everything kept "transposed" so each stage's
output is the next stage's natural PE operand:
  qkv^T [768, 2048] = (w_qkvT tiles).T @ xT tiles          (j on partitions)
  rope on q/k tiles via pair-swap permutation matmul + elementwise tables
  S^T [s, t] = KrT_tile.T @ QrT  -> exp on ACT -> P^T bf16 (causal: skip/mask)
  r[t] = ones.T @ P^T (row sums, broadcast across partitions)
  U^T [d, t] = V_tile.T-accumulated PV; AO^T = U^T * (1/r)
  OUT [t, m] = AO^T_tile.T @ w_oT -> fp32 partial to DRAM
"""
import numpy as np
import ml_dtypes

T, DIM, H, HKV, D, NCORES = 2048, 4096, 32, 8, 128, 8
HL = H // NCORES            # 4 local q heads
JL = (HL + 2) * D           # 768 local qkv rows
WO_L = HL * D               # 512 local w_o cols
SCALE = float(D) ** -0.5
THETA = 10000.0
NP_BF16 = ml_dtypes.bfloat16

_CACHE = {}


def _build_nc(reps=1):
    from contextlib import ExitStack
    import concourse.bacc as bacc
    import concourse.mybir as mybir
    from concourse.tile import TileContext
    from concourse.masks import make_identity

    bf = mybir.dt.bfloat16
    f32 = mybir.dt.float32
    Exp = mybir.ActivationFunctionType.Exp

    nc = bacc.Bacc("TRN2", target_bir_lowering=False, debug=False,
                   num_devices=NCORES)
    xT_h = nc.dram_tensor("xT", [DIM, T], bf, kind="ExternalInput")
    wq_h = nc.dram_tensor("wqkvT", [DIM, JL], bf, kind="ExternalInput")
    wo_h = nc.dram_tensor("woT", [WO_L, DIM], bf, kind="ExternalInput")
    cos_h = nc.dram_tensor("cosb", [D, T], f32, kind="ExternalInput")
    sin_h = nc.dram_tensor("sinb", [D, T], f32, kind="ExternalInput")
    psw_h = nc.dram_tensor("pswap", [D, D], bf, kind="ExternalInput")
    msk_h = nc.dram_tensor("mask01", [D, D], bf, kind="ExternalInput")
    out_h = nc.dram_tensor("outp", [T, DIM], f32, kind="ExternalOutput")
    xT, wq, wo = xT_h.ap(), wq_h.ap(), wo_h.ap()
    cosb, sinb, psw, msk, outp = cos_h.ap(), sin_h.ap(), psw_h.ap(), msk_h.ap(), out_h.ap()

    def emit_once(tc, top):
        const = top.enter_context(tc.tile_pool(name="const", bufs=1))
        pswap_sb = const.tile([D, D], bf, name="pswap_sb")
        mask_sb = const.tile([D, D], bf, name="mask_sb")
        ones_sb = const.tile([D, D], bf, name="ones_sb")
        ident_sb = const.tile([D, D], bf, name="ident_sb")
        nc.sync.dma_start(pswap_sb[:], psw[:, :])
        nc.sync.dma_start(mask_sb[:], msk[:, :])
        nc.vector.memset(ones_sb[:], 1.0)
        make_identity(nc, ident_sb[:])

        persist = top.enter_context(tc.tile_pool(name="persist", bufs=1))
        # Qr^T per local head + Kr^T: bf16 [128, T]
        rot_sb = [persist.tile([D, T], bf, name=f"rot{jt}", tag=f"rot{jt}")
                  for jt in range(HL + 1)]
        # V natural orientation, one [128, 128] tile per s-block
        v_sb = [persist.tile([D, D], bf, name=f"v{st}", tag=f"v{st}")
                for st in range(T // D)]
        # AO^T per local head
        ao_sb = [persist.tile([D, T], bf, name=f"ao{h}", tag=f"ao{h}")
                 for h in range(HL)]

        # ---------------- phase 1: QKV + RoPE + V transpose ----------------
        with ExitStack() as ph1:
            wp = ph1.enter_context(tc.tile_pool(name="wp", bufs=1))
            xp = ph1.enter_context(tc.tile_pool(name="xp", bufs=1))
            rp = ph1.enter_context(tc.tile_pool(name="rp", bufs=1))
            sp = ph1.enter_context(tc.tile_pool(name="sp", bufs=2))
            tp = ph1.enter_context(tc.tile_pool(name="tp", bufs=4))
            qps = ph1.enter_context(tc.tile_pool(name="qps", bufs=2, space="PSUM"))
            sps = ph1.enter_context(tc.tile_pool(name="sps", bufs=2, space="PSUM"))
            vps = ph1.enter_context(tc.tile_pool(name="vps", bufs=2, space="PSUM"))

            TH = 1024  # t-half width
            # x and w DMAs interleaved in PE consumption order so the first
            # QKV sweep can start streaming as soon as its operands land
            w_sb = []
            x0_sb = []
            for i in range(32):
                xt = xp.tile([D, TH], bf, name=f"x{i}", tag=f"x{i}")
                nc.sync.dma_start(xt[:], xT[i * 128:(i + 1) * 128, 0:TH])
                x0_sb.append(xt)
                wt = wp.tile([D, JL], bf, name=f"w{i}", tag=f"w{i}")
                nc.sync.dma_start(wt[:], wq[i * 128:(i + 1) * 128, :])
                w_sb.append(wt)
            cos_sb = rp.tile([D, T], f32, name="cos_sb")
            sin_sb = rp.tile([D, T], f32, name="sin_sb")
            nc.sync.dma_start(cos_sb[:], cosb[:, :])
            nc.sync.dma_start(sin_sb[:], sinb[:, :])

            for th in range(2):
                t0 = th * TH
                if th == 0:
                    x_sb = x0_sb
                else:
                    x_sb = []
                    for i in range(32):
                        xt = xp.tile([D, TH], bf, name=f"x{i}", tag=f"x{i}")
                        nc.sync.dma_start(xt[:], xT[i * 128:(i + 1) * 128, t0:t0 + TH])
                        x_sb.append(xt)
                for jt in range(6):
                    qkv_ps = qps.tile([D, TH], f32, name="qkv_ps", tag="qkv")
                    for i in range(32):
                        lhsT = w_sb[i][:, jt * 128:(jt + 1) * 128]
                        for t2 in range(2):
                            nc.tensor.matmul(
                                qkv_ps[:, t2 * 512:(t2 + 1) * 512], lhsT,
                                x_sb[i][:, t2 * 512:(t2 + 1) * 512],
                                start=(i == 0), stop=(i == 31))
                    qkv_bf = sp.tile([D, TH], bf, name="qkv_bf", tag="qkv_bf")
                    nc.vector.tensor_copy(qkv_bf[:], qkv_ps[:])
                    if jt < 5:
                        # rope: rot = qkv*cos + (P qkv)*sin_signed
                        for t2 in range(2):
                            sl = slice(t2 * 512, (t2 + 1) * 512)
                            gl = slice(t0 + t2 * 512, t0 + t2 * 512 + 512)
                            sw_ps = sps.tile([D, 512], f32, name="sw_ps", tag="sw")
                            nc.tensor.matmul(sw_ps[:], pswap_sb[:], qkv_bf[:, sl],
                                             start=True, stop=True)
                            t1 = tp.tile([D, 512], f32, name="t1", tag="t1")
                            t2t = tp.tile([D, 512], f32, name="t2t", tag="t2t")
                            nc.vector.tensor_mul(t1[:], qkv_bf[:, sl], cos_sb[:, gl])
                            nc.vector.tensor_mul(t2t[:], sw_ps[:], sin_sb[:, gl])
                            nc.vector.tensor_add(rot_sb[jt][:, gl], t1[:], t2t[:])
                    else:
                        # V: transpose [t, d] blocks out of v^T
                        for k in range(8):
                            st = th * 8 + k
                            vt_ps = vps.tile([D, D], bf, name="vt_ps", tag="vt")
                            nc.tensor.transpose(
                                vt_ps[:], qkv_bf[:, k * 128:(k + 1) * 128],
                                ident_sb[:])
                            nc.vector.tensor_copy(v_sb[st][:], vt_ps[:])

        # w_o prefetch pool opened before phase 2 so its DMAs overlap attention
        wop = top.enter_context(tc.tile_pool(name="wop", bufs=1))
        wo_sb = []
        for jc in range(HL):
            wt = wop.tile([D, DIM], bf, name=f"wo{jc}", tag=f"wo{jc}")
            nc.sync.dma_start(wt[:], wo[jc * 128:(jc + 1) * 128, :])
            wo_sb.append(wt)

        # ---------------- phase 2: causal attention per head ----------------
        with ExitStack() as ph2:
            ptp = ph2.enter_context(tc.tile_pool(name="ptp", bufs=20))
            rip = ph2.enter_context(tc.tile_pool(name="rip", bufs=2))
            scs = ph2.enter_context(tc.tile_pool(name="scs", bufs=4, space="PSUM"))
            rps = ph2.enter_context(tc.tile_pool(name="rps", bufs=2, space="PSUM"))
            ups = ph2.enter_context(tc.tile_pool(name="ups", bufs=2, space="PSUM"))

            for h in range(HL):
                for j in range(4):          # t-chunks of 512
                    n_st = 4 * j + 4
                    pts = []
                    for st in range(n_st):
                        t_off = max(0, st - 4 * j) * 128
                        sc = scs.tile([D, 512], f32, name="sc", tag="sc")
                        nc.tensor.matmul(
                            sc[:, t_off:], rot_sb[HL][:, st * 128:(st + 1) * 128],
                            rot_sb[h][:, j * 512 + t_off:(j + 1) * 512],
                            start=True, stop=True)
                        pt = ptp.tile([D, 512], bf, name="pt", tag="pt")
                        nc.scalar.activation(pt[:, t_off:], sc[:, t_off:], Exp,
                                             scale=SCALE)
                        if st >= 4 * j:
                            nc.vector.tensor_mul(pt[:, t_off:t_off + 128],
                                                 pt[:, t_off:t_off + 128],
                                                 mask_sb[:])
                        pts.append(pt)
                    r_ps = rps.tile([D, 512], f32, name="r_ps", tag="r")
                    u_ps = ups.tile([D, 512], f32, name="u_ps", tag="u")
                    for st in range(n_st):
                        t_off = max(0, st - 4 * j) * 128
                        nc.tensor.matmul(r_ps[:, t_off:], ones_sb[:],
                                         pts[st][:, t_off:],
                                         start=(st == 0), stop=(st == n_st - 1))
                    for st in range(n_st):
                        t_off = max(0, st - 4 * j) * 128
                        nc.tensor.matmul(u_ps[:, t_off:], v_sb[st][:],
                                         pts[st][:, t_off:],
                                         start=(st == 0), stop=(st == n_st - 1))
                    r_inv = rip.tile([D, 512], f32, name="r_inv", tag="ri")
                    nc.vector.reciprocal(r_inv[:], r_ps[:])
                    nc.vector.tensor_mul(ao_sb[h][:, j * 512:(j + 1) * 512],
                                         u_ps[:], r_inv[:])

        # ---------------- phase 3: output projection (partial) ----------------
        with ExitStack() as ph3:
            obp = ph3.enter_context(tc.tile_pool(name="obp", bufs=3))
            ops = ph3.enter_context(tc.tile_pool(name="ops", bufs=2, space="PSUM"))
            for tt in range(16):
                for half in range(2):
                    po = ops.tile([D, 2048], f32, name="po", tag="po")
                    for jc in range(HL):
                        lhsT = ao_sb[jc][:, tt * 128:(tt + 1) * 128]
                        for mb in range(4):
                            m0 = half * 2048 + mb * 512
                            nc.tensor.matmul(po[:, mb * 512:(mb + 1) * 512],
                                             lhsT, wo_sb[jc][:, m0:m0 + 512],
                                             start=(jc == 0), stop=(jc == HL - 1))
                    ob = obp.tile([D, 2048], f32, name="ob", tag="ob")
                    nc.vector.tensor_copy(ob[:], po[:])
                    nc.sync.dma_start(
                        outp[tt * 128:(tt + 1) * 128,
                             half * 2048:(half + 1) * 2048], ob[:])

    from contextlib import ExitStack as _ES
    with TileContext(nc) as tc:
        for _rep in range(reps):
            with _ES() as top:
                emit_once(tc, top)

    nc.compile()
    return nc


def get_nc(reps=1):
    key = ("nc", reps)
    if key not in _CACHE:
        _CACHE[key] = _build_nc(reps)
    return _CACHE[key]


def host_prep(x, w_qkv, w_o):
    """Returns per-core input maps (numpy)."""
    x = np.asarray(x)
    w_qkv = np.asarray(w_qkv)
    w_o = np.asarray(w_o)
    xT = np.ascontiguousarray(x.T)
    inv_freq = 1.0 / (THETA ** (np.arange(0, D, 2, dtype=np.float64) / D))
    ang = np.arange(T, dtype=np.float64)[:, None] * inv_freq[None, :]
    cosb = np.empty((D, T), np.float32)
    sinb = np.empty((D, T), np.float32)
    cosb[0::2] = np.cos(ang).T
    cosb[1::2] = np.cos(ang).T
    sinb[0::2] = -np.sin(ang).T
    sinb[1::2] = np.sin(ang).T
    pswap = np.zeros((D, D), NP_BF16)
    for d in range(D):
        pswap[d, d ^ 1] = 1
    mask01 = np.triu(np.ones((128, 128), np.float32)).astype(NP_BF16)
    in_maps = []
    for c in range(NCORES):
        wq_rows = w_qkv[c * HL * D:(c + 1) * HL * D]
        wk_rows = w_qkv[H * D + c * D: H * D + (c + 1) * D]
        wv_rows = w_qkv[(H + HKV) * D + c * D:(H + HKV) * D + (c + 1) * D]
        w_c = np.concatenate([wq_rows, wk_rows, wv_rows], axis=0)
        in_maps.append({
            "xT": xT,
            "wqkvT": np.ascontiguousarray(w_c.T),
            "woT": np.ascontiguousarray(w_o[:, c * WO_L:(c + 1) * WO_L].T),
            "cosb": cosb, "sinb": sinb, "pswap": pswap, "mask01": mask01,
        })
    return in_maps


def kernel(x, w_qkv, w_o):
    from concourse.bass_utils import run_bass_kernel_spmd
    nc = get_nc()
    in_maps = host_prep(x, w_qkv, w_o)
    res = run_bass_kernel_spmd(nc, in_maps, list(range(NCORES)))
    acc = np.zeros((T, DIM), np.float32)
    for c in range(NCORES):
        acc += res.results[c]["outp"]
    return acc.astype(NP_BF16)
